# revision 30
# baseline (speedup 1.0000x reference)
"""Trainium2 Bass kernel for the additive-attention layer.

Math (per batch b):
    pre[s, h]   = enc[b] @ W2 + hidden[b] @ W1 + b_attn      (W1=W_attn[:H], W2=W_attn[H:])
    energy      = tanh(pre)
    scores[s]   = energy @ w_v (+ b_v, irrelevant: softmax is shift-invariant)
    attn        = softmax(scores)
    context     = attn @ enc[b]

Distribution: data-parallel over batch, 4 batches per core, no collectives.

Design (driven by NTFF traces on the 8-core axon setup):
  - The PE does ONLY the 2048 main matmuls, back-to-back at 216ns/MM
    (2.4GHz warm).  Scores are computed off-PE: per j-group a DVE
    tensor_scalar_mul (energy * w_v[h]) accumulates into a bf16 running
    sum; one GpSimd partition_all_reduce per chunk yields the scores row
    broadcast to all 128 partitions.  This removes 128 scores matmuls
    AND the per-group weight-buffer eviction they caused (~0.3us/group).
  - exp runs on the full [128,512] broadcast (all rows identical), so its
    output IS the partition-broadcast exp(p) needed by the context muls;
    accum_out row 0 gives the chunk denominator.
  - hproj = W1.T@hidden.T + b_attn computed on the HOST (8 MFLOP, 16KB
    shipped) — removes 2MB w1 from the startup critical path.
  - enc ships as fp8 e3m4 scaled x2 (bf16 W2 stationary; fp8 moving runs
    at bf16 speed, exact e10m11 upcast).  Halves tunnel upload + HBM
    traffic + SBUF; measured accuracy on the graded inputs: ctx 1.17e-2 /
    attn 0.73e-2 vs the 2e-2 gate (bf16 fallback: fp8=False, 2.1e-3).
    Crucially it also keeps all 8 cores at 2.4GHz: dense bf16 on 8 cores
    trips the chip power limit and downclocks the PE to 2.0GHz.
  - Host layout is chunk-major (b, c, p, k, s): each 1MB chunk DMA is one
    contiguous 8KB run per partition.  w2 is split into two tiles with the
    first enc chunk DMA'd between them (single HWDGE FIFO — order matters)
    so the first matmul group only waits for 3MB.
  - ~36 dependency-free warm-up matmuls on a zeroed tile keep the PE
    HAM-warm until real operands land (else the first ~3.4us run at
    1.2GHz).
  - Softmax normalization (the divide) happens on the HOST: the device
    ships raw exp rows, per-chunk denominators, and context numerators —
    no device-side reciprocal/broadcast/scale tail.
  - ctx numerators: DVE mul (et * exp) then per-k free-dim reduce, split
    DVE tensor_reduce / ACT Identity+accum_out (act_split) for balance.
"""

import numpy as np
import ml_dtypes
from contextlib import ExitStack

import concourse.bacc as bacc
import concourse.bass as bass
import concourse.tile as tile
import concourse.mybir as mybir
from concourse.bass import bass_isa
from concourse.bass_utils import run_bass_kernel_spmd

B, S, H = 32, 2048, 1024
D = 2 * H                     # encoder feature dim
NCORES = 8
BPC = B // NCORES             # batches per core
SCH = 512                     # s-chunk (one PSUM bank of fp32)
NCH = S // SCH
NDT = D // 128                # d-tiles (contraction tiles for main matmul)
NHT = H // 128                # h-tiles

BF16 = mybir.dt.bfloat16
F32 = mybir.dt.float32
FP8 = mybir.dt.float8e3
FP8_NP = ml_dtypes.float8_e3m4
FP8_SCALE = 2.0
FP8_MAX = 15.5

_CACHE = {}


def _build(reps=1, bench_mode=False, fp8=True, act_split=10,
           encp_bufs=4, enp_bufs=12, ppre_bufs=6, scr_bufs=4, pbc_bufs=3,
           warmup=30, ablate=""):
    # ablate: comma-set of {"noctx", "noscores"} for bench ablations
    nc = bacc.Bacc("TRN2", target_bir_lowering=False, debug=False)
    enc_dt = FP8 if fp8 else BF16
    inv_scale = (1.0 / FP8_SCALE) if fp8 else 1.0

    # bench_mode: big inputs become device-resident Internal tensors
    # (garbage data) so repeated timed executions don't ship 150MB through
    # the axon tunnel; engine timing is data-independent.
    kind = "Internal" if bench_mode else "ExternalInput"
    # chunk-major: (b, c, p, k, s) — one contiguous run per partition/chunk
    encT = nc.dram_tensor("enct", (BPC, NCH, 128, NDT, SCH), enc_dt, kind=kind).ap()
    w2 = nc.dram_tensor("w2", (D, H), BF16, kind=kind).ap()
    hp = nc.dram_tensor("hproj", (H, BPC), F32, kind=kind).ap()
    wv = nc.dram_tensor("wv", (H,), BF16, kind=kind).ap()
    # raw softmax numerators + per-chunk denominators; host normalizes
    ctx_out = nc.dram_tensor("ctx", (BPC, D), F32, kind="ExternalOutput").ap()
    attn_out = nc.dram_tensor("attn", (BPC, S), F32, kind="ExternalOutput").ap()
    den_out = nc.dram_tensor("den", (1, BPC * NCH), F32, kind="ExternalOutput").ap()
    warmsink = nc.dram_tensor("warmsink", (128, 4), F32, kind="Internal").ap()

    with tile.TileContext(nc) as tc, ExitStack() as ctx:
        weights = ctx.enter_context(tc.tile_pool(name="weights", bufs=1))
        encp = ctx.enter_context(tc.tile_pool(name="encp", bufs=encp_bufs))
        enp = ctx.enter_context(tc.tile_pool(name="enp", bufs=enp_bufs))
        small = ctx.enter_context(tc.tile_pool(name="small", bufs=1))
        bcp = ctx.enter_context(tc.tile_pool(name="bcp", bufs=pbc_bufs))
        scr = ctx.enter_context(tc.tile_pool(name="scr", bufs=scr_bufs))
        ppre = ctx.enter_context(tc.tile_pool(name="ppre", bufs=ppre_bufs, space="PSUM"))
        pwarm = ctx.enter_context(tc.tile_pool(name="pwarm", bufs=1, space="PSUM"))

        # --- PE warm-up: dependency-free matmuls on a zeroed tile keep the
        # PE HAM-warm (2.4GHz) until the first real operands land ---
        if warmup:
            wz = small.tile([128, SCH], BF16, name="warmzero")
            nc.vector.memset(wz, 0.0)
            wp = pwarm.tile([128, SCH], F32)
            for _ in range(warmup):
                nc.tensor.matmul(wp, wz[:, :128], wz, start=True, stop=True)
            ws = small.tile([128, 4], F32, name="warmout")
            nc.vector.tensor_copy(ws, wp[:, :4])
            nc.sync.dma_start(out=warmsink, in_=ws)

        # --- resident tensors; order on the single HWDGE FIFO matters ---
        hp_sb = small.tile([128, NHT, BPC], F32)
        nc.sync.dma_start(out=hp_sb, in_=hp.rearrange("(j p) b -> p j b", p=128))
        wv_sb = small.tile([128, NHT], BF16)
        nc.sync.dma_start(out=wv_sb, in_=wv.rearrange("(j p) -> p j", p=128))
        wv_f32 = small.tile([128, NHT], F32)
        nc.vector.tensor_copy(wv_f32, wv_sb)  # tensor_scalar needs f32 scalar
        NQ = NHT // 4  # j-groups per w2 quarter
        w2_q = []
        for q in range(4):
            wq = weights.tile([128, NDT, NQ * 128], BF16, tag=f"w2q{q}")
            w2_q.append(wq)
            nc.sync.dma_start(
                out=wq,
                in_=w2[:, q * NQ * 128:(q + 1) * NQ * 128].rearrange(
                    "(k p) h -> p k h", p=128
                ),
            )
            if q == 0:
                # first enc chunk right after the first w2 quarter: the
                # first matmul group then only waits for 2MB on the FIFO
                et0 = encp.tile([128, NDT, SCH], enc_dt, tag="et")
                nc.sync.dma_start(out=et0, in_=encT[0, 0])

        def w2_slice(k, j):
            jj = j % NQ
            return w2_q[j // NQ][:, k, jj * 128:(jj + 1) * 128]

        for _rep in range(reps):
            denAll = small.tile([128, BPC * NCH], F32, name="denAll", tag="denAll")
            # raw context numerator partials, column layout (b, k, c)
            ctxp = small.tile([128, BPC * NDT * NCH], F32, name="ctxp", tag="ctxp")
            if ablate:
                nc.vector.memset(ctxp, 0.0)
                nc.vector.memset(denAll, 1.0)

            for b in range(BPC):
                for c in range(NCH):
                    last_chunk = (b == BPC - 1) and (c == NCH - 1)
                    if _rep == 0 and b == 0 and c == 0:
                        et = et0
                    else:
                        et = encp.tile([128, NDT, SCH], enc_dt, tag="et")
                        nc.sync.dma_start(out=et, in_=encT[b, c])
                    et_mul = et
                    if last_chunk and fp8:
                        # the last chunk's ctx muls are the exposed tail:
                        # a bf16 shadow (SWDGE cast-DMA, runs during this
                        # chunk's compute) + bf16 exp copy gets DVE 2x mode
                        et_mul = encp.tile([128, NDT, SCH], BF16, tag="etlast")
                        nc.gpsimd.dma_start(out=et_mul, in_=encT[b, c])

                    acc = None
                    for j in range(NHT):
                        pp = ppre.tile([128, SCH], F32)
                        for k in range(NDT):
                            nc.tensor.matmul(
                                pp,
                                w2_slice(k, j),
                                et[:, k, :],
                                start=(k == 0),
                                stop=(k == NDT - 1),
                            )
                        en = enp.tile([128, SCH], BF16)
                        nc.scalar.activation(
                            out=en,
                            in_=pp,
                            func=mybir.ActivationFunctionType.Tanh,
                            bias=hp_sb[:, j, b:b + 1],
                            scale=inv_scale,
                        )
                        if "noscores" in ablate:
                            continue
                        # scores contribution on DVE: sp[h,s] = en * w_v[h]
                        sp = scr.tile([128, SCH], BF16, tag="sp")
                        nc.vector.tensor_scalar_mul(sp, en, wv_f32[:, j:j + 1])
                        if acc is None:
                            acc = sp
                        else:
                            acc2 = scr.tile([128, SCH], BF16, tag="acc")
                            nc.vector.tensor_add(acc2, acc, sp)
                            acc = acc2

                    if "noscores" in ablate:
                        continue
                    # scores row (broadcast to all partitions) via GpSimd
                    ar = bcp.tile([128, SCH], F32, tag="ar")
                    nc.gpsimd.partition_all_reduce(
                        ar, acc, channels=128, reduce_op=bass_isa.ReduceOp.add
                    )
                    # exp of the broadcast: row 0 is the attn row, the full
                    # tile is the partition-broadcast weights for ctx, and
                    # accum row 0 is the chunk denominator
                    pbc = bcp.tile([128, SCH], F32, tag="pbc")
                    dcol = b * NCH + c
                    nc.scalar.activation(
                        out=pbc,
                        in_=ar,
                        func=mybir.ActivationFunctionType.Exp,
                        accum_out=denAll[:, dcol:dcol + 1],
                    )
                    nc.sync.dma_start(
                        out=attn_out[b:b + 1, c * SCH:(c + 1) * SCH],
                        in_=pbc[0:1, :],
                    )

                    if "noctx" in ablate:
                        continue
                    pbc_m = pbc
                    if last_chunk and fp8:
                        pbc_m = scr.tile([128, SCH], BF16, tag="pbcbf")
                        nc.vector.tensor_copy(pbc_m, pbc)
                    for k in range(NDT):
                        col = (b * NDT + k) * NCH + c
                        prod = scr.tile([128, SCH], BF16, tag="prod")
                        nc.vector.tensor_mul(prod, et_mul[:, k, :], pbc_m)
                        if k < act_split:
                            prod2 = scr.tile([128, SCH], BF16, tag="prod2")
                            nc.scalar.activation(
                                out=prod2,
                                in_=prod,
                                func=mybir.ActivationFunctionType.Identity,
                                accum_out=ctxp[:, col:col + 1],
                            )
                        else:
                            nc.vector.tensor_reduce(
                                ctxp[:, col:col + 1],
                                prod,
                                axis=mybir.AxisListType.X,
                                op=mybir.AluOpType.add,
                            )
                    if c == NCH - 1:
                        # batch b complete: reduce chunk partials and ship
                        ctxr = bcp.tile([128, NDT], F32, tag="ctxr")
                        nc.vector.tensor_reduce(
                            ctxr,
                            ctxp[:, b * NDT * NCH:(b + 1) * NDT * NCH].rearrange(
                                "p (x c) -> p x c", c=NCH
                            ),
                            axis=mybir.AxisListType.X,
                            op=mybir.AluOpType.add,
                        )
                        nc.sync.dma_start(
                            out=ctx_out[b].rearrange("(k p) -> p k", p=128),
                            in_=ctxr,
                        )
                        nc.sync.dma_start(
                            out=den_out[0:1, b * NCH:(b + 1) * NCH],
                            in_=denAll[0:1, b * NCH:(b + 1) * NCH],
                        )

    nc.compile()
    return nc


FP8_DEFAULT = True


def _get_nc():
    if "nc" not in _CACHE:
        _CACHE["nc"] = _build(fp8=FP8_DEFAULT)
    return _CACHE["nc"]


def _prep_inputs(hidden, encoder_outputs, W_attn, b_attn, w_v, b_v, fp8=True):
    bf16 = ml_dtypes.bfloat16
    W1 = W_attn[:H]
    w2 = np.ascontiguousarray(W_attn[H:]).astype(bf16)
    wv_ = w_v.astype(bf16)
    # hproj = W1.T @ hidden.T + b_attn on the host: (H, B) f32
    hproj_all = (hidden.astype(np.float32) @ W1.astype(np.float32)).T \
        + np.asarray(b_attn, np.float32)[:, None]
    if fp8:
        encq = np.clip(encoder_outputs * FP8_SCALE, -FP8_MAX, FP8_MAX).astype(FP8_NP)
    else:
        encq = encoder_outputs.astype(bf16)
    in_maps = []
    for core in range(NCORES):
        sl = slice(core * BPC, (core + 1) * BPC)
        # (b, s, d) -> chunk-major (b, c, p, k, s')
        encT = np.ascontiguousarray(
            encq[sl].reshape(BPC, NCH, SCH, NDT, 128).transpose(0, 1, 4, 3, 2)
        )
        in_maps.append(
            {
                "enct": encT,
                "w2": w2,
                "hproj": np.ascontiguousarray(hproj_all[:, sl]),
                "wv": wv_,
            }
        )
    return in_maps


def kernel(hidden, encoder_outputs, W_attn, b_attn, w_v, b_v, _trace=False):
    nc = _get_nc()
    fp8 = FP8_DEFAULT
    in_maps = _prep_inputs(hidden, encoder_outputs, W_attn, b_attn, w_v, b_v, fp8=fp8)
    res = run_bass_kernel_spmd(
        nc, in_maps, core_ids=list(range(NCORES)), trace=_trace
    )
    # ctx numerators carry the fp8 x2 scale; fold it into the denominator
    ctx_den_scale = FP8_SCALE if fp8 else 1.0
    ctxs, attns = [], []
    for r in res.results:
        den = r["den"].reshape(BPC, NCH).sum(axis=1)          # (BPC,)
        attns.append(r["attn"] / den[:, None])
        ctxs.append(r["ctx"] / (ctx_den_scale * den[:, None]))
    context = np.concatenate(ctxs, axis=0)
    attn = np.concatenate(attns, axis=0)
    if _trace:
        _CACHE["last_results"] = res
    return context, attn


# revision 36
# speedup vs baseline: 1.0281x; 1.0281x over previous
"""Trainium2 Bass kernel for the additive-attention layer.

Math (per batch b):
    pre[s, h]   = enc[b] @ W2 + hidden[b] @ W1 + b_attn      (W1=W_attn[:H], W2=W_attn[H:])
    energy      = tanh(pre)
    scores[s]   = energy @ w_v (+ b_v, irrelevant: softmax is shift-invariant)
    attn        = softmax(scores)
    context     = attn @ enc[b]

Distribution: data-parallel over batch, 4 batches per core, no collectives.

Design (driven by NTFF traces on the 8-core axon setup):
  - The PE does ONLY the 2048 main matmuls, back-to-back at 216ns/MM
    (2.4GHz warm).  Scores are computed off-PE: per j-group a DVE
    tensor_scalar_mul (energy * w_v[h]) accumulates into a bf16 running
    sum; one GpSimd partition_all_reduce per chunk yields the scores row
    broadcast to all 128 partitions.  This removes 128 scores matmuls
    AND the per-group weight-buffer eviction they caused (~0.3us/group).
  - exp runs on the full [128,512] broadcast (all rows identical), so its
    output IS the partition-broadcast exp(p) needed by the context muls;
    accum_out row 0 gives the chunk denominator.
  - hproj = W1.T@hidden.T + b_attn computed on the HOST (8 MFLOP, 16KB
    shipped) — removes 2MB w1 from the startup critical path.
  - enc ships as fp8 e3m4 scaled x2 (bf16 W2 stationary; fp8 moving runs
    at bf16 speed, exact e10m11 upcast).  Halves tunnel upload + HBM
    traffic + SBUF; measured accuracy on the graded inputs: ctx 1.17e-2 /
    attn 0.73e-2 vs the 2e-2 gate (bf16 fallback: fp8=False, 2.1e-3).
    Crucially it also keeps all 8 cores at 2.4GHz: dense bf16 on 8 cores
    trips the chip power limit and downclocks the PE to 2.0GHz.
  - Host layout is chunk-major (b, c, p, k, s): each 1MB chunk DMA is one
    contiguous 8KB run per partition.  w2 is split into two tiles with the
    first enc chunk DMA'd between them (single HWDGE FIFO — order matters)
    so the first matmul group only waits for 3MB.
  - ~36 dependency-free warm-up matmuls on a zeroed tile keep the PE
    HAM-warm until real operands land (else the first ~3.4us run at
    1.2GHz).
  - Softmax normalization (the divide) happens on the HOST: the device
    ships raw exp rows, per-chunk denominators, and context numerators —
    no device-side reciprocal/broadcast/scale tail.
  - ctx numerators: DVE mul (et * exp) then per-k free-dim reduce, split
    DVE tensor_reduce / ACT Identity+accum_out (act_split) for balance.
"""

import numpy as np
import ml_dtypes
from contextlib import ExitStack

import concourse.bacc as bacc
import concourse.bass as bass
import concourse.tile as tile
import concourse.mybir as mybir
from concourse.bass import bass_isa
from concourse.bass_utils import run_bass_kernel_spmd

B, S, H = 32, 2048, 1024
D = 2 * H                     # encoder feature dim
NCORES = 8
BPC = B // NCORES             # batches per core
SCH = 512                     # s-chunk (one PSUM bank of fp32)
NCH = S // SCH
NDT = D // 128                # d-tiles (contraction tiles for main matmul)
NHT = H // 128                # h-tiles

BF16 = mybir.dt.bfloat16
F32 = mybir.dt.float32
FP8 = mybir.dt.float8e3
FP8_NP = ml_dtypes.float8_e3m4
FP8_SCALE = 2.0
FP8_MAX = 15.5

_CACHE = {}


def _build(reps=1, bench_mode=False, fp8=True, act_split=10,
           encp_bufs=4, enp_bufs=12, ppre_bufs=6, scr_bufs=4, pbc_bufs=3,
           warmup=30, ablate=""):
    # ablate: comma-set of {"noctx", "noscores"} for bench ablations
    nc = bacc.Bacc("TRN2", target_bir_lowering=False, debug=False)
    enc_dt = FP8 if fp8 else BF16
    inv_scale = (1.0 / FP8_SCALE) if fp8 else 1.0

    # bench_mode: big inputs become device-resident Internal tensors
    # (garbage data) so repeated timed executions don't ship 150MB through
    # the axon tunnel; engine timing is data-independent.
    kind = "Internal" if bench_mode else "ExternalInput"
    # ALL dram layouts are host-pre-swizzled so every DMA is contiguous per
    # partition — AP-rearrange DMAs shatter into 4-byte descriptors that
    # take ~10us to drain (measured on the ctx output)
    # chunk-major: (b, c, p, k, s) — one contiguous run per partition/chunk
    encT = nc.dram_tensor("enct", (BPC, NCH, 128, NDT, SCH), enc_dt, kind=kind).ap()
    w2 = nc.dram_tensor("w2", (128, 4, NDT, H // 4), BF16, kind=kind).ap()
    hp = nc.dram_tensor("hproj", (128, NHT, BPC), F32, kind=kind).ap()
    wv = nc.dram_tensor("wv", (128, NHT), BF16, kind=kind).ap()
    # raw softmax numerators + per-chunk denominators; host normalizes and
    # un-swizzles ctx (b, p, k) -> (b, k*128+p)
    ctx_out = nc.dram_tensor("ctx", (BPC, 128, NDT), F32, kind="ExternalOutput").ap()
    attn_out = nc.dram_tensor("attn", (BPC, S), F32, kind="ExternalOutput").ap()
    den_out = nc.dram_tensor("den", (1, BPC * NCH), F32, kind="ExternalOutput").ap()
    warmsink = nc.dram_tensor("warmsink", (128, 4), F32, kind="Internal").ap()

    with tile.TileContext(nc) as tc, ExitStack() as ctx:
        weights = ctx.enter_context(tc.tile_pool(name="weights", bufs=1))
        encp = ctx.enter_context(tc.tile_pool(name="encp", bufs=encp_bufs))
        enp = ctx.enter_context(tc.tile_pool(name="enp", bufs=enp_bufs))
        small = ctx.enter_context(tc.tile_pool(name="small", bufs=1))
        bcp = ctx.enter_context(tc.tile_pool(name="bcp", bufs=pbc_bufs))
        scr = ctx.enter_context(tc.tile_pool(name="scr", bufs=scr_bufs))
        ppre = ctx.enter_context(tc.tile_pool(name="ppre", bufs=ppre_bufs, space="PSUM"))
        pwarm = ctx.enter_context(tc.tile_pool(name="pwarm", bufs=1, space="PSUM"))

        # --- PE warm-up: dependency-free matmuls on a zeroed tile keep the
        # PE HAM-warm (2.4GHz) until the first real operands land ---
        if warmup:
            wz = small.tile([128, SCH], BF16, name="warmzero")
            nc.vector.memset(wz, 0.0)
            wp = pwarm.tile([128, SCH], F32)
            for _ in range(warmup):
                nc.tensor.matmul(wp, wz[:, :128], wz, start=True, stop=True)
            ws = small.tile([128, 4], F32, name="warmout")
            nc.vector.tensor_copy(ws, wp[:, :4])
            nc.sync.dma_start(out=warmsink, in_=ws)

        # --- resident tensors; order on the single HWDGE FIFO matters ---
        hp_sb = small.tile([128, NHT, BPC], F32)
        nc.sync.dma_start(out=hp_sb, in_=hp)
        wv_sb = small.tile([128, NHT], BF16)
        nc.sync.dma_start(out=wv_sb, in_=wv)
        wv_f32 = small.tile([128, NHT], F32)
        nc.vector.tensor_copy(wv_f32, wv_sb)  # tensor_scalar needs f32 scalar
        NQ = NHT // 4  # j-groups per w2 quarter
        w2_q = []
        for q in range(4):
            wq = weights.tile([128, NDT, NQ * 128], BF16, tag=f"w2q{q}")
            w2_q.append(wq)
            nc.sync.dma_start(out=wq, in_=w2[:, q])
            if q == 0:
                # first enc chunk right after the first w2 quarter: the
                # first matmul group then only waits for 2MB on the FIFO
                et0 = encp.tile([128, NDT, SCH], enc_dt, tag="et")
                nc.sync.dma_start(out=et0, in_=encT[0, 0])

        def w2_slice(k, j):
            jj = j % NQ
            return w2_q[j // NQ][:, k, jj * 128:(jj + 1) * 128]

        for _rep in range(reps):
            denAll = small.tile([128, BPC * NCH], F32, name="denAll", tag="denAll")
            # raw context numerator partials, column layout (b, k, c)
            ctxp = small.tile([128, BPC * NDT * NCH], F32, name="ctxp", tag="ctxp")
            if ablate:
                nc.vector.memset(ctxp, 0.0)
                nc.vector.memset(denAll, 1.0)

            for b in range(BPC):
                for c in range(NCH):
                    last_chunk = (b == BPC - 1) and (c == NCH - 1)
                    if _rep == 0 and b == 0 and c == 0:
                        et = et0
                    else:
                        et = encp.tile([128, NDT, SCH], enc_dt, tag="et")
                        nc.sync.dma_start(out=et, in_=encT[b, c])
                    et_mul = et
                    if last_chunk and fp8:
                        # the last chunk's ctx muls are the exposed tail:
                        # a bf16 shadow (SWDGE cast-DMA, runs during this
                        # chunk's compute) + bf16 exp copy gets DVE 2x mode
                        et_mul = encp.tile([128, NDT, SCH], BF16, tag="etlast")
                        nc.gpsimd.dma_start(out=et_mul, in_=encT[b, c])

                    acc = None
                    for j in range(NHT):
                        pp = ppre.tile([128, SCH], F32)
                        for k in range(NDT):
                            nc.tensor.matmul(
                                pp,
                                w2_slice(k, j),
                                et[:, k, :],
                                start=(k == 0),
                                stop=(k == NDT - 1),
                            )
                        en = enp.tile([128, SCH], BF16)
                        nc.scalar.activation(
                            out=en,
                            in_=pp,
                            func=mybir.ActivationFunctionType.Tanh,
                            bias=hp_sb[:, j, b:b + 1],
                            scale=inv_scale,
                        )
                        if "noscores" in ablate:
                            continue
                        # scores contribution on DVE: sp[h,s] = en * w_v[h]
                        sp = scr.tile([128, SCH], BF16, tag="sp")
                        nc.vector.tensor_scalar_mul(sp, en, wv_f32[:, j:j + 1])
                        if acc is None:
                            acc = sp
                        else:
                            acc2 = scr.tile([128, SCH], BF16, tag="acc")
                            nc.vector.tensor_add(acc2, acc, sp)
                            acc = acc2

                    if "noscores" in ablate:
                        continue
                    # scores row (broadcast to all partitions) via GpSimd
                    ar = bcp.tile([128, SCH], F32, tag="ar")
                    nc.gpsimd.partition_all_reduce(
                        ar, acc, channels=128, reduce_op=bass_isa.ReduceOp.add
                    )
                    # exp of the broadcast: row 0 is the attn row, the full
                    # tile is the partition-broadcast weights for ctx, and
                    # accum row 0 is the chunk denominator
                    pbc = bcp.tile([128, SCH], F32, tag="pbc")
                    dcol = b * NCH + c
                    nc.scalar.activation(
                        out=pbc,
                        in_=ar,
                        func=mybir.ActivationFunctionType.Exp,
                        accum_out=denAll[:, dcol:dcol + 1],
                    )
                    nc.sync.dma_start(
                        out=attn_out[b:b + 1, c * SCH:(c + 1) * SCH],
                        in_=pbc[0:1, :],
                    )

                    if "noctx" in ablate:
                        continue
                    pbc_m = pbc
                    if last_chunk and fp8:
                        pbc_m = scr.tile([128, SCH], BF16, tag="pbcbf")
                        nc.vector.tensor_copy(pbc_m, pbc)
                    for k in range(NDT):
                        col = (b * NDT + k) * NCH + c
                        prod = scr.tile([128, SCH], BF16, tag="prod")
                        nc.vector.tensor_mul(prod, et_mul[:, k, :], pbc_m)
                        if k < act_split:
                            prod2 = scr.tile([128, SCH], BF16, tag="prod2")
                            nc.scalar.activation(
                                out=prod2,
                                in_=prod,
                                func=mybir.ActivationFunctionType.Identity,
                                accum_out=ctxp[:, col:col + 1],
                            )
                        else:
                            nc.vector.tensor_reduce(
                                ctxp[:, col:col + 1],
                                prod,
                                axis=mybir.AxisListType.X,
                                op=mybir.AluOpType.add,
                            )
                    if c == NCH - 1:
                        # batch b complete: reduce chunk partials and ship
                        ctxr = bcp.tile([128, NDT], F32, tag="ctxr")
                        nc.vector.tensor_reduce(
                            ctxr,
                            ctxp[:, b * NDT * NCH:(b + 1) * NDT * NCH].rearrange(
                                "p (x c) -> p x c", c=NCH
                            ),
                            axis=mybir.AxisListType.X,
                            op=mybir.AluOpType.add,
                        )
                        nc.sync.dma_start(out=ctx_out[b], in_=ctxr)
                        nc.sync.dma_start(
                            out=den_out[0:1, b * NCH:(b + 1) * NCH],
                            in_=denAll[0:1, b * NCH:(b + 1) * NCH],
                        )

    nc.compile()
    return nc


FP8_DEFAULT = True


def _get_nc():
    if "nc" not in _CACHE:
        _CACHE["nc"] = _build(fp8=FP8_DEFAULT)
    return _CACHE["nc"]


def _prep_inputs(hidden, encoder_outputs, W_attn, b_attn, w_v, b_v, fp8=True):
    bf16 = ml_dtypes.bfloat16
    W1 = W_attn[:H]
    # w2 pre-swizzled to (128, quarter, NDT, 256): contiguous per partition
    w2 = np.ascontiguousarray(
        W_attn[H:].astype(bf16).reshape(NDT, 128, 4, H // 4).transpose(1, 2, 0, 3)
    )
    wv_ = np.ascontiguousarray(w_v.astype(bf16).reshape(NHT, 128).T)
    # hproj = W1.T @ hidden.T + b_attn on the host, shipped as (128, j, b)
    hproj_all = (hidden.astype(np.float32) @ W1.astype(np.float32)).T \
        + np.asarray(b_attn, np.float32)[:, None]
    if fp8:
        encq = np.clip(encoder_outputs * FP8_SCALE, -FP8_MAX, FP8_MAX).astype(FP8_NP)
    else:
        encq = encoder_outputs.astype(bf16)
    in_maps = []
    for core in range(NCORES):
        sl = slice(core * BPC, (core + 1) * BPC)
        # (b, s, d) -> chunk-major (b, c, p, k, s')
        encT = np.ascontiguousarray(
            encq[sl].reshape(BPC, NCH, SCH, NDT, 128).transpose(0, 1, 4, 3, 2)
        )
        hp_core = np.ascontiguousarray(
            hproj_all[:, sl].reshape(NHT, 128, BPC).transpose(1, 0, 2)
        )
        in_maps.append(
            {
                "enct": encT,
                "w2": w2,
                "hproj": hp_core,
                "wv": wv_,
            }
        )
    return in_maps


def kernel(hidden, encoder_outputs, W_attn, b_attn, w_v, b_v, _trace=False):
    nc = _get_nc()
    fp8 = FP8_DEFAULT
    in_maps = _prep_inputs(hidden, encoder_outputs, W_attn, b_attn, w_v, b_v, fp8=fp8)
    res = run_bass_kernel_spmd(
        nc, in_maps, core_ids=list(range(NCORES)), trace=_trace
    )
    # ctx numerators carry the fp8 x2 scale; fold it into the denominator
    ctx_den_scale = FP8_SCALE if fp8 else 1.0
    ctxs, attns = [], []
    for r in res.results:
        den = r["den"].reshape(BPC, NCH).sum(axis=1)          # (BPC,)
        attns.append(r["attn"] / den[:, None])
        # un-swizzle (b, p, k) -> (b, d = k*128+p)
        ctx_raw = r["ctx"].transpose(0, 2, 1).reshape(BPC, D)
        ctxs.append(ctx_raw / (ctx_den_scale * den[:, None]))
    context = np.concatenate(ctxs, axis=0)
    attn = np.concatenate(attns, axis=0)
    if _trace:
        _CACHE["last_results"] = res
    return context, attn
